# revision 31
# baseline (speedup 1.0000x reference)
"""MPNCOV (iSQRT-COV pooling) Trainium2 kernel — v3.

Math per sample (C=256 channels, M=196 spatial):
  xc   = x - mean_m(x)                      # center along spatial dim
  A    = xc @ xc^T / sum(xc^2)              # = cov / trace(cov)
  Newton-Schulz (ITER_N=3) on A, final y = sqrt(normA) * YZY, triu-packed.

Scale folding (sigma tracked symbolically; every PSUM->SBUF transform is one
512-wide op):
  A_s   = G * (1/tr)          (gram G unnormalized; 1/tr folded into drain)
  ZY1_s = 3I   - A_s          Y1_s  = A_s @ ZY1_s
  ZY2_s = 12I  - ZY1_s@Y1_s   Y2_s  = Y1_s @ ZY2_s   Z2_s = ZY2_s @ ZY1_s
  ZY3_s = 768I - Z2_s@Y2_s    F_s   = Y2_s @ ZY3_s
  y = (sqrt(tr/M)/8192) * F_s

Schedule: 4 waves x 8 samples, stage-major. Within a wave each NS stage runs
as 8 back-to-back PE matmul groups so the tensor engine stays continuously
busy and holds its full 2.4 GHz p-state (an intermittently-idle PE runs at
1.2 GHz and every matmul costs 2x). PSUM drains trail on DVE/ACT; GpSimd is
used ONLY for the partition allreduce (its per-op launch overhead makes it
useless for general elementwise) and for flush DMA issue. The next wave's
loads and input statistics are interleaved into this wave's NS stage
emission so no engine queue runs dry.

Output: fp16 ftile/[S,NTRIU] fp16 DRAM tensor (host upcasts to f32); flush
is one DMA per matrix row r covering all 32 samples (256 DMAs, 2 or 3 dim
APs), round-robined over the three DMA-capable engines (sync/scalar HWDGE,
gpsimd SWDGE).

Sharding: pure data parallel, batch 256 -> 32 samples on each of 8 cores.
"""

import contextlib

import numpy as np

from concourse import bacc, bass, bass_isa, mybir, tile
from concourse import bass_utils

F32 = mybir.dt.float32
F16 = mybir.dt.float16
P = 128
C = 256
M = 196
B = 256
NCORES = 8
S = B // NCORES            # samples per core
NTRIU = C * (C + 1) // 2   # 32896
W = 8                      # samples per wave

MM_DT = mybir.dt.float16

LAST_EXEC_NS = None
LAST_RESULTS = None

ROWSTART = np.concatenate([[0], np.cumsum(C - np.arange(C))]).astype(np.int64)


def build(tc, y_ap, x_ap, ident_ap, icons_ap, n_samples=S):
    nc = tc.nc

    with contextlib.ExitStack() as ctx:
        consts = ctx.enter_context(tc.tile_pool(name="consts", bufs=1))
        fpool = ctx.enter_context(tc.tile_pool(name="fpool", bufs=1))
        work = ctx.enter_context(tc.tile_pool(name="work", bufs=1))
        mats = ctx.enter_context(tc.tile_pool(name="mats", bufs=1))
        psum = ctx.enter_context(tc.tile_pool(name="psum", bufs=5, space="PSUM"))
        tpsum = ctx.enter_context(tc.tile_pool(name="tpsum", bufs=3, space="PSUM"))

        ident = consts.tile([P, P], MM_DT, tag="ident")
        nc.sync.dma_start(ident[:], ident_ap[:])
        icons = consts.tile([P, 3, 2 * C], MM_DT, tag="icons")
        nc.sync.dma_start(icons[:], icons_ap[:])

        # fp16 result tile: cols 0:256 = F rows 0:128, cols 256:512 = 128:256
        ftile = fpool.tile([P, n_samples, 2 * C], F16, tag="F", name="F")

        state = [dict() for _ in range(n_samples)]

        def prod(U, V):
            """One [128,512] fp32 PSUM bank <- U @ V ([P,512] fp16, symmetric)."""
            p_t = psum.tile([P, 2 * C], F32, tag="pp", name="pp")
            for mt in range(2):
                oc = slice(mt * C, (mt + 1) * C)
                ms0 = slice(mt * P, mt * P + P)
                ms1 = slice(C + mt * P, C + mt * P + P)
                nc.tensor.matmul(
                    p_t[:, oc], U[:, ms0], V[:, 0:C], start=True, stop=False
                )
                nc.tensor.matmul(
                    p_t[:, oc], U[:, ms1], V[:, C : 2 * C], start=False, stop=True
                )
            return p_t

        def load(b):
            x = state[b]
            fx = f"_{b % W}"
            x["xr"] = work.tile([P, 2, M], F32, tag="xr" + fx, name="xr" + fx)
            nc.sync.dma_start(x["xr"][:], x_ap[b].rearrange("(h p) m -> p h m", p=P))

        def stats(b):
            """Mean + centering only; the trace comes later from the gram
            diagonal (one DVE tensor_tensor_reduce) instead of Square+accum
            passes over the input."""
            x = state[b]
            fx = f"_{b % W}"
            xr = x["xr"]
            mean2 = work.tile([P, 2], F32, tag="mean2" + fx, name="mean2" + fx)
            nc.vector.tensor_reduce(
                mean2[:], xr[:], axis=mybir.AxisListType.X, op=mybir.AluOpType.add
            )
            negmean = work.tile([P, 2], F32, tag="negmean" + fx, name="nm" + fx)
            nc.vector.tensor_scalar_mul(negmean[:], mean2[:], -1.0 / M)
            xc = work.tile([P, 2, M], MM_DT, tag="xc" + fx, name="xc" + fx)
            nc.vector.tensor_scalar_add(xc[:, 0], xr[:, 0], negmean[:, 0:1])
            nc.vector.tensor_scalar_add(xc[:, 1], xr[:, 1], negmean[:, 1:2])
            sqd = work.tile([P, 2, M], F16, tag="sqd" + fx, name="sqd" + fx)
            s2 = work.tile([P, 2], F32, tag="s2" + fx, name="s2" + fx)
            for h in range(2):
                nc.scalar.activation(
                    sqd[:, h], xr[:, h],
                    mybir.ActivationFunctionType.Square,
                    bias=negmean[:, h : h + 1],
                    accum_out=s2[:, h : h + 1],
                )
            s2r = work.tile([P, 2], F32, tag="s2r" + fx, name="s2r" + fx)
            nc.gpsimd.partition_all_reduce(
                s2r[:], s2[:], channels=P, reduce_op=bass_isa.ReduceOp.add
            )
            trv = work.tile([P, 1], F32, tag="trv" + fx, name="trv" + fx)
            nc.vector.tensor_tensor(
                trv[:], s2r[:, 0:1], s2r[:, 1:2], op=mybir.AluOpType.add
            )
            itr = work.tile([P, 1], F32, tag="itr" + fx, name="itr" + fx)
            nc.vector.reciprocal(itr[:], trv[:])
            abv1 = work.tile([P, 1], F32, tag="abv1" + fx, name="abv1" + fx)
            nc.scalar.activation(
                abv1[:], trv[:], mybir.ActivationFunctionType.Sqrt,
                scale=1.0 / (M * 8192.0 * 8192.0),
            )
            x["xc"], x["itr"], x["abv1"] = xc, itr, abv1

        def transpose(b):
            x = state[b]
            xc = x["xc"]
            tp = tpsum.tile([P, 2 * C], MM_DT, tag="tp", name="tp")
            for h in range(2):
                nc.tensor.transpose(tp[:, h * P : h * P + P], xc[:, h, 0:P], ident[:])
                nc.tensor.transpose(
                    tp[0 : M - P, C + h * P : C + h * P + P], xc[:, h, P:M], ident[:]
                )
            x["tp"] = tp

        def drain_tp(b):
            """Plain copies (normalization by 1/tr is folded into drain_A)."""
            x = state[b]
            fx = f"_{b % W}"
            tp = x["tp"]
            xcT0 = work.tile([P, C], MM_DT, tag="xcT0" + fx, name="xcT0" + fx)
            xcT1 = work.tile([P, C], MM_DT, tag="xcT1" + fx, name="xcT1" + fx)
            nc.vector.tensor_scalar_mul(xcT0[:], tp[:, 0:C], 1.0)
            nc.scalar.activation(
                xcT1[0 : M - P], tp[0 : M - P, C : 2 * C],
                mybir.ActivationFunctionType.Copy,
            )
            x["xcT0"], x["xcT1"] = xcT0, xcT1

        def gram(b):
            x = state[b]
            xcT0, xcT1 = x["xcT0"], x["xcT1"]
            a_ps = psum.tile([P, 2 * C], F32, tag="pp", name="aps")
            for mt in range(2):
                oc = slice(mt * C, (mt + 1) * C)
                ms = slice(mt * P, (mt + 1) * P)
                nc.tensor.matmul(
                    a_ps[:, oc], xcT0[:, ms], xcT0[:], start=True, stop=False
                )
                nc.tensor.matmul(
                    a_ps[:, oc], xcT1[0 : M - P, ms], xcT1[0 : M - P, :],
                    start=False, stop=True,
                )
            x["a_ps"] = a_ps

        def mat(b, tag):
            fx = f"_{b % W}"
            t = mats.tile([P, 2 * C], MM_DT, tag=tag + fx, name=tag + fx)
            state[b][tag] = t
            return t

        def drain_A(b):
            # A_s = G * (1/tr): normalization folded into the PSUM drain
            nc.scalar.activation(
                mat(b, "A")[:], state[b]["a_ps"][:],
                mybir.ActivationFunctionType.Copy, scale=state[b]["itr"][:, 0:1],
            )

        def zy1(b):
            nc.vector.tensor_tensor(
                mat(b, "ZY1")[:], icons[:, 0, :], state[b]["A"][:],
                op=mybir.AluOpType.subtract,
            )

        def mm(b, dst, u, v):
            state[b][dst] = prod(state[b][u], state[b][v])

        def drain(b, dst, src, eng):
            t = mat(b, dst)
            if eng == "act":
                nc.scalar.activation(
                    t[:], state[b][src][:], mybir.ActivationFunctionType.Copy
                )
            else:
                nc.vector.tensor_scalar_mul(t[:], state[b][src][:], 1.0)

        def sub(b, dst, k, src):
            nc.vector.tensor_tensor(
                mat(b, dst)[:], icons[:, k, :], state[b][src][:],
                op=mybir.AluOpType.subtract,
            )

        def fstore(b):
            x = state[b]
            nc.vector.tensor_scalar_mul(
                ftile[:, b, :], x["f_ps"][:], x["abv1"][:, 0:1]
            )

        def ns_stages(b):
            return [
                (lambda: transpose(b), lambda: drain_tp(b)),
                (lambda: gram(b),
                 lambda: (drain_A(b), zy1(b))),
                (lambda: mm(b, "y1_ps", "A", "ZY1"),
                 lambda: drain(b, "Y1", "y1_ps", "act")),
                (lambda: mm(b, "w1_ps", "ZY1", "Y1"),
                 lambda: sub(b, "ZY2", 1, "w1_ps")),
                (lambda: mm(b, "y2_ps", "Y1", "ZY2"),
                 lambda: drain(b, "Y2", "y2_ps", "dve")),
                (lambda: mm(b, "z2_ps", "ZY2", "ZY1"),
                 lambda: drain(b, "Z2", "z2_ps", "act")),
                (lambda: mm(b, "w2_ps", "Z2", "Y2"),
                 lambda: sub(b, "ZY3", 2, "w2_ps")),
                (lambda: mm(b, "f_ps", "Y2", "ZY3"), lambda: fstore(b)),
            ]

        def flush_row(r, eng):
            L = C - r
            s0 = int(ROWSTART[r])
            # tile col of matrix col m for row-half h = h*C + m
            cs = (r // P) * C + r
            src = ftile[r % P : r % P + 1, :, cs : cs + L]
            eng.dma_start(y_ap[:, s0 : s0 + L], src)

        waves = [list(range(w0, min(w0 + W, n_samples)))
                 for w0 in range(0, n_samples, W)]

        for b in waves[0]:
            load(b)
        for b in waves[0]:
            stats(b)

        for wi, wave in enumerate(waves):
            nxt = waves[wi + 1] if wi + 1 < len(waves) else []
            for b in nxt:
                load(b)
            stages = [ns_stages(b) for b in wave]
            nstage = len(stages[0])
            for si in range(nstage):
                for sg in stages:
                    sg[si][0]()
                for sg in stages:
                    sg[si][1]()
                for b in nxt[si * len(nxt) // nstage
                             : (si + 1) * len(nxt) // nstage]:
                    stats(b)

        # ---- flush: one DMA per matrix row, all samples at once ----
        engines = [nc.sync, nc.scalar, nc.gpsimd]
        for r in range(C):
            flush_row(r, engines[r % 3])

def _make_const_inputs():
    e = np.zeros((P, 2 * C), np.float32)
    e[np.arange(P), np.arange(P)] = 1.0
    e[np.arange(P), C + P + np.arange(P)] = 1.0
    icons = np.stack([3.0 * e, 12.0 * e, 768.0 * e], axis=1).astype(np.float16)
    return {
        "ident": np.eye(P, dtype=np.float16),
        "icons": np.ascontiguousarray(icons),
    }


def make_nc(n_samples=S, num_devices=NCORES):
    nc = bacc.Bacc(
        "TRN2",
        target_bir_lowering=False,
        debug=False,
        enable_asserts=False,
        num_devices=num_devices,
    )
    x_ap = nc.dram_tensor("x", (n_samples, C, M), F32, kind="ExternalInput").ap()
    y_ap = nc.dram_tensor("y", (n_samples, NTRIU), F16, kind="ExternalOutput").ap()
    ident_ap = nc.dram_tensor("ident", (P, P), MM_DT, kind="ExternalInput").ap()
    icons_ap = nc.dram_tensor("icons", (P, 3, 2 * C), MM_DT, kind="ExternalInput").ap()
    with tile.TileContext(nc) as tc:
        build(tc, y_ap, x_ap, ident_ap, icons_ap, n_samples)
    nc.compile()
    return nc


def kernel(x, _trace=False, **_trace_kwargs):
    global LAST_EXEC_NS, LAST_RESULTS
    x = np.ascontiguousarray(np.asarray(x), dtype=np.float32)
    assert x.shape == (B, C, 14, 14)
    xr = x.reshape(B, C, M)

    nc = make_nc()
    consts = _make_const_inputs()
    in_maps = [
        {"x": np.ascontiguousarray(xr[i * S : (i + 1) * S]), **consts}
        for i in range(NCORES)
    ]
    res = bass_utils.run_bass_kernel_spmd(
        nc, in_maps, core_ids=list(range(NCORES)), trace=_trace, **_trace_kwargs
    )
    LAST_EXEC_NS = res.exec_time_ns
    LAST_RESULTS = res
    return np.concatenate(
        [r["y"].astype(np.float32) for r in res.results], axis=0
    )


# revision 32
# speedup vs baseline: 1.0121x; 1.0121x over previous
"""MPNCOV (iSQRT-COV pooling) Trainium2 kernel — v3.

Math per sample (C=256 channels, M=196 spatial):
  xc   = x - mean_m(x)                      # center along spatial dim
  A    = xc @ xc^T / sum(xc^2)              # = cov / trace(cov)
  Newton-Schulz (ITER_N=3) on A, final y = sqrt(normA) * YZY, triu-packed.

Scale folding (sigma tracked symbolically; every PSUM->SBUF transform is one
512-wide op):
  A_s   = G * (1/tr)          (gram G unnormalized; 1/tr folded into drain)
  ZY1_s = 3I   - A_s          Y1_s  = A_s @ ZY1_s
  ZY2_s = 12I  - ZY1_s@Y1_s   Y2_s  = Y1_s @ ZY2_s   Z2_s = ZY2_s @ ZY1_s
  ZY3_s = 768I - Z2_s@Y2_s    F_s   = Y2_s @ ZY3_s
  y = (sqrt(tr/M)/8192) * F_s

Schedule: 4 waves x 8 samples, stage-major. Within a wave each NS stage runs
as 8 back-to-back PE matmul groups so the tensor engine stays continuously
busy and holds its full 2.4 GHz p-state (an intermittently-idle PE runs at
1.2 GHz and every matmul costs 2x). PSUM drains trail on DVE/ACT; GpSimd is
used ONLY for the partition allreduce (its per-op launch overhead makes it
useless for general elementwise) and for flush DMA issue. The next wave's
loads and input statistics are interleaved into this wave's NS stage
emission so no engine queue runs dry.

Output: fp16 ftile/[S,NTRIU] fp16 DRAM tensor (host upcasts to f32); flush
is one DMA per matrix row r covering all 32 samples (256 DMAs, 2 or 3 dim
APs), round-robined over the three DMA-capable engines (sync/scalar HWDGE,
gpsimd SWDGE).

Sharding: pure data parallel, batch 256 -> 32 samples on each of 8 cores.
"""

import contextlib

import numpy as np

from concourse import bacc, bass, bass_isa, mybir, tile
from concourse import bass_utils

F32 = mybir.dt.float32
F16 = mybir.dt.float16
P = 128
C = 256
M = 196
B = 256
NCORES = 8
S = B // NCORES            # samples per core
NTRIU = C * (C + 1) // 2   # 32896
W = 8                      # samples per wave

MM_DT = mybir.dt.float16

LAST_EXEC_NS = None
LAST_RESULTS = None

ROWSTART = np.concatenate([[0], np.cumsum(C - np.arange(C))]).astype(np.int64)


def build(tc, y_ap, x_ap, ident_ap, icons_ap, n_samples=S):
    nc = tc.nc

    with contextlib.ExitStack() as ctx:
        consts = ctx.enter_context(tc.tile_pool(name="consts", bufs=1))
        fpool = ctx.enter_context(tc.tile_pool(name="fpool", bufs=1))
        work = ctx.enter_context(tc.tile_pool(name="work", bufs=1))
        mats = ctx.enter_context(tc.tile_pool(name="mats", bufs=1))
        psum = ctx.enter_context(tc.tile_pool(name="psum", bufs=6, space="PSUM"))
        tpsum = ctx.enter_context(tc.tile_pool(name="tpsum", bufs=2, space="PSUM"))

        ident = consts.tile([P, P], MM_DT, tag="ident")
        nc.sync.dma_start(ident[:], ident_ap[:])
        icons = consts.tile([P, 3, 2 * C], MM_DT, tag="icons")
        nc.sync.dma_start(icons[:], icons_ap[:])

        # fp16 result tile: cols 0:256 = F rows 0:128, cols 256:512 = 128:256
        ftile = fpool.tile([P, n_samples, 2 * C], F16, tag="F", name="F")

        state = [dict() for _ in range(n_samples)]

        def prod(U, V):
            """One [128,512] fp32 PSUM bank <- U @ V ([P,512] fp16, symmetric)."""
            p_t = psum.tile([P, 2 * C], F32, tag="pp", name="pp")
            for mt in range(2):
                oc = slice(mt * C, (mt + 1) * C)
                ms0 = slice(mt * P, mt * P + P)
                ms1 = slice(C + mt * P, C + mt * P + P)
                nc.tensor.matmul(
                    p_t[:, oc], U[:, ms0], V[:, 0:C], start=True, stop=False
                )
                nc.tensor.matmul(
                    p_t[:, oc], U[:, ms1], V[:, C : 2 * C], start=False, stop=True
                )
            return p_t

        def load(b):
            x = state[b]
            fx = f"_{b % W}"
            x["xr"] = work.tile([P, 2, M], F32, tag="xr" + fx, name="xr" + fx)
            nc.sync.dma_start(x["xr"][:], x_ap[b].rearrange("(h p) m -> p h m", p=P))

        def stats(b):
            """Mean + centering only; the trace comes later from the gram
            diagonal (one DVE tensor_tensor_reduce) instead of Square+accum
            passes over the input."""
            x = state[b]
            fx = f"_{b % W}"
            xr = x["xr"]
            mean2 = work.tile([P, 2], F32, tag="mean2" + fx, name="mean2" + fx)
            nc.vector.tensor_reduce(
                mean2[:], xr[:], axis=mybir.AxisListType.X, op=mybir.AluOpType.add
            )
            negmean = work.tile([P, 2], F32, tag="negmean" + fx, name="nm" + fx)
            nc.vector.tensor_scalar_mul(negmean[:], mean2[:], -1.0 / M)
            xc = work.tile([P, 2, M], MM_DT, tag="xc" + fx, name="xc" + fx)
            nc.vector.tensor_scalar_add(xc[:, 0], xr[:, 0], negmean[:, 0:1])
            nc.vector.tensor_scalar_add(xc[:, 1], xr[:, 1], negmean[:, 1:2])
            sqd = work.tile([P, 2, M], F16, tag="sqd" + fx, name="sqd" + fx)
            s2 = work.tile([P, 2], F32, tag="s2" + fx, name="s2" + fx)
            for h in range(2):
                nc.scalar.activation(
                    sqd[:, h], xr[:, h],
                    mybir.ActivationFunctionType.Square,
                    bias=negmean[:, h : h + 1],
                    accum_out=s2[:, h : h + 1],
                )
            s2r = work.tile([P, 2], F32, tag="s2r" + fx, name="s2r" + fx)
            nc.gpsimd.partition_all_reduce(
                s2r[:], s2[:], channels=P, reduce_op=bass_isa.ReduceOp.add
            )
            trv = work.tile([P, 1], F32, tag="trv" + fx, name="trv" + fx)
            nc.vector.tensor_tensor(
                trv[:], s2r[:, 0:1], s2r[:, 1:2], op=mybir.AluOpType.add
            )
            itr = work.tile([P, 1], F32, tag="itr" + fx, name="itr" + fx)
            nc.vector.reciprocal(itr[:], trv[:])
            abv1 = work.tile([P, 1], F32, tag="abv1" + fx, name="abv1" + fx)
            nc.scalar.activation(
                abv1[:], trv[:], mybir.ActivationFunctionType.Sqrt,
                scale=1.0 / (M * 8192.0 * 8192.0),
            )
            x["xc"], x["itr"], x["abv1"] = xc, itr, abv1

        def transpose(b):
            x = state[b]
            xc = x["xc"]
            tp = tpsum.tile([P, 2 * C], MM_DT, tag="tp", name="tp")
            for h in range(2):
                nc.tensor.transpose(tp[:, h * P : h * P + P], xc[:, h, 0:P], ident[:])
                nc.tensor.transpose(
                    tp[0 : M - P, C + h * P : C + h * P + P], xc[:, h, P:M], ident[:]
                )
            x["tp"] = tp

        def drain_tp(b):
            """Plain copies (normalization by 1/tr is folded into drain_A)."""
            x = state[b]
            fx = f"_{b % W}"
            tp = x["tp"]
            xcT0 = work.tile([P, C], MM_DT, tag="xcT0" + fx, name="xcT0" + fx)
            xcT1 = work.tile([P, C], MM_DT, tag="xcT1" + fx, name="xcT1" + fx)
            nc.vector.tensor_scalar_mul(xcT0[:], tp[:, 0:C], 1.0)
            nc.scalar.activation(
                xcT1[0 : M - P], tp[0 : M - P, C : 2 * C],
                mybir.ActivationFunctionType.Copy,
            )
            x["xcT0"], x["xcT1"] = xcT0, xcT1

        def gram(b):
            x = state[b]
            xcT0, xcT1 = x["xcT0"], x["xcT1"]
            a_ps = psum.tile([P, 2 * C], F32, tag="pp", name="aps")
            for mt in range(2):
                oc = slice(mt * C, (mt + 1) * C)
                ms = slice(mt * P, (mt + 1) * P)
                nc.tensor.matmul(
                    a_ps[:, oc], xcT0[:, ms], xcT0[:], start=True, stop=False
                )
                nc.tensor.matmul(
                    a_ps[:, oc], xcT1[0 : M - P, ms], xcT1[0 : M - P, :],
                    start=False, stop=True,
                )
            x["a_ps"] = a_ps

        def mat(b, tag):
            fx = f"_{b % W}"
            t = mats.tile([P, 2 * C], MM_DT, tag=tag + fx, name=tag + fx)
            state[b][tag] = t
            return t

        def drain_A(b):
            # A_s = G * (1/tr): normalization folded into the PSUM drain
            nc.scalar.activation(
                mat(b, "A")[:], state[b]["a_ps"][:],
                mybir.ActivationFunctionType.Copy, scale=state[b]["itr"][:, 0:1],
            )

        def zy1(b):
            nc.vector.tensor_tensor(
                mat(b, "ZY1")[:], icons[:, 0, :], state[b]["A"][:],
                op=mybir.AluOpType.subtract,
            )

        def mm(b, dst, u, v):
            state[b][dst] = prod(state[b][u], state[b][v])

        def drain(b, dst, src, eng):
            t = mat(b, dst)
            if eng == "act":
                nc.scalar.activation(
                    t[:], state[b][src][:], mybir.ActivationFunctionType.Copy
                )
            else:
                nc.vector.tensor_scalar_mul(t[:], state[b][src][:], 1.0)

        def sub(b, dst, k, src):
            nc.vector.tensor_tensor(
                mat(b, dst)[:], icons[:, k, :], state[b][src][:],
                op=mybir.AluOpType.subtract,
            )

        def fstore(b):
            x = state[b]
            nc.vector.tensor_scalar_mul(
                ftile[:, b, :], x["f_ps"][:], x["abv1"][:, 0:1]
            )

        def ns_stages(b):
            return [
                (lambda: transpose(b), lambda: drain_tp(b)),
                (lambda: gram(b),
                 lambda: (drain_A(b), zy1(b))),
                (lambda: mm(b, "y1_ps", "A", "ZY1"),
                 lambda: drain(b, "Y1", "y1_ps", "act")),
                (lambda: mm(b, "w1_ps", "ZY1", "Y1"),
                 lambda: sub(b, "ZY2", 1, "w1_ps")),
                (lambda: mm(b, "y2_ps", "Y1", "ZY2"),
                 lambda: drain(b, "Y2", "y2_ps", "dve")),
                (lambda: mm(b, "z2_ps", "ZY2", "ZY1"),
                 lambda: drain(b, "Z2", "z2_ps", "act")),
                (lambda: mm(b, "w2_ps", "Z2", "Y2"),
                 lambda: sub(b, "ZY3", 2, "w2_ps")),
                (lambda: mm(b, "f_ps", "Y2", "ZY3"), lambda: fstore(b)),
            ]

        def flush_row(r, eng):
            L = C - r
            s0 = int(ROWSTART[r])
            # tile col of matrix col m for row-half h = h*C + m
            cs = (r // P) * C + r
            src = ftile[r % P : r % P + 1, :, cs : cs + L]
            eng.dma_start(y_ap[:, s0 : s0 + L], src)

        waves = [list(range(w0, min(w0 + W, n_samples)))
                 for w0 in range(0, n_samples, W)]

        for b in waves[0]:
            load(b)
        for b in waves[0]:
            stats(b)

        for wi, wave in enumerate(waves):
            nxt = waves[wi + 1] if wi + 1 < len(waves) else []
            for b in nxt:
                load(b)
            stages = [ns_stages(b) for b in wave]
            nstage = len(stages[0])
            for si in range(nstage):
                for sg in stages:
                    sg[si][0]()
                for sg in stages:
                    sg[si][1]()
                for b in nxt[si * len(nxt) // nstage
                             : (si + 1) * len(nxt) // nstage]:
                    stats(b)

        # ---- flush: one DMA per matrix row, all samples at once ----
        engines = [nc.sync, nc.scalar, nc.gpsimd]
        for r in range(C):
            flush_row(r, engines[r % 3])

def _make_const_inputs():
    e = np.zeros((P, 2 * C), np.float32)
    e[np.arange(P), np.arange(P)] = 1.0
    e[np.arange(P), C + P + np.arange(P)] = 1.0
    icons = np.stack([3.0 * e, 12.0 * e, 768.0 * e], axis=1).astype(np.float16)
    return {
        "ident": np.eye(P, dtype=np.float16),
        "icons": np.ascontiguousarray(icons),
    }


def make_nc(n_samples=S, num_devices=NCORES):
    nc = bacc.Bacc(
        "TRN2",
        target_bir_lowering=False,
        debug=False,
        enable_asserts=False,
        num_devices=num_devices,
    )
    x_ap = nc.dram_tensor("x", (n_samples, C, M), F32, kind="ExternalInput").ap()
    y_ap = nc.dram_tensor("y", (n_samples, NTRIU), F16, kind="ExternalOutput").ap()
    ident_ap = nc.dram_tensor("ident", (P, P), MM_DT, kind="ExternalInput").ap()
    icons_ap = nc.dram_tensor("icons", (P, 3, 2 * C), MM_DT, kind="ExternalInput").ap()
    with tile.TileContext(nc) as tc:
        build(tc, y_ap, x_ap, ident_ap, icons_ap, n_samples)
    nc.compile()
    return nc


def kernel(x, _trace=False, **_trace_kwargs):
    global LAST_EXEC_NS, LAST_RESULTS
    x = np.ascontiguousarray(np.asarray(x), dtype=np.float32)
    assert x.shape == (B, C, 14, 14)
    xr = x.reshape(B, C, M)

    nc = make_nc()
    consts = _make_const_inputs()
    in_maps = [
        {"x": np.ascontiguousarray(xr[i * S : (i + 1) * S]), **consts}
        for i in range(NCORES)
    ]
    res = bass_utils.run_bass_kernel_spmd(
        nc, in_maps, core_ids=list(range(NCORES)), trace=_trace, **_trace_kwargs
    )
    LAST_EXEC_NS = res.exec_time_ns
    LAST_RESULTS = res
    return np.concatenate(
        [r["y"].astype(np.float32) for r in res.results], axis=0
    )
